# revision 7
# baseline (speedup 1.0000x reference)
"""Trainium2 Bass kernel for BaselineBlockNetSingleGraph (GRU + attention-GCN + convs + big linear).

Sharding: data-parallel over batch B=64 across 8 cores (8 batches/core) for
everything up to the final linear; the final linear's 196608-wide reduction is
column-sharded across cores (24576 each) via an on-device AllToAll of the
activations (bf16), with an AllReduce of the [64, 768] partials.

v2 restructure: since the GCN aggregation (n-axis), merged gcn+conv (c,w axes)
are commuting linear maps, per-block order alternates so each block needs only
ONE layout flip, done by strided DMA through DRAM (no PE transposes):
  block0: conv0 (CP) -> flip -> agg0+bias+lrelu (NP)
  block1: agg1 (NP) -> flip -> conv1+bias+lrelu (CP, padded)
  block2: conv2 (CP) -> flip -> agg2+bias+lrelu (NP) -> A2A send
All conv/agg matmuls in bf16. Final linear uses DMA XBAR transposes for the
[64,k]->[k,64] activation tiles instead of PE transposes.

On-chip layouts (per core; plane = batch half, plane0 = local batches 0-3):
  CP: [128 = (plane, c), (n=64, wp=54)] per local batch (w padded by 3 both sides)
  NP: [128 = (plane, n), (c=64, w=48)] per local batch
"""

import os
import numpy as np
import ml_dtypes

import concourse.bass as bass
import concourse.tile as tile
from concourse import mybir, bacc
from concourse.bass_utils import run_bass_kernel_spmd

F32 = mybir.dt.float32
F32R = mybir.dt.float32r
BF16 = mybir.dt.bfloat16
AF = mybir.ActivationFunctionType
ALU = mybir.AluOpType

B, W, N, C, H, QK, HOR = 64, 48, 64, 64, 64, 32, 12
NCORES = 8
BL = B // NCORES          # 8 local batches
BP = BL // 2              # 4 batches per plane
SEQ = BL * N              # 512 sequences per core
WP = W + 6                # padded w
KCH = W * N * C // NCORES # 24576 reduction chunk per core
KS = [3, 5, 7]
TOFF = [0, 3, 8]
CPF = BP * N * WP         # 13824
RO = N * HOR              # 768
NWF = N * W               # 3072 per-batch flat (n,w) or (c,w)
PLS = 64 * BP * NWF       # 786432: plane stride in flip scratch


def _ap(base_ap, off, dims):
    """AP with same partition dim as base_ap, extra element offset, given free dims."""
    return bass.AP(tensor=base_ap.tensor, offset=base_ap.offset + off,
                   ap=[list(base_ap.ap[0])] + [list(d) for d in dims])


def _rap(base_ap, off, dims):
    """Raw AP on a DRAM tile: explicit dims incl. the partition-paired one."""
    return bass.AP(tensor=base_ap.tensor, offset=base_ap.offset + off,
                   ap=[list(d) for d in dims])


def _build():
    nc = bacc.Bacc("TRN2", target_bir_lowering=False, debug=False, num_devices=NCORES)
    P = nc.declare_dram_parameter

    x1 = P("x1", [1, W * SEQ], BF16, isOutput=False)
    state0 = P("state0", [64, SEQ], F32R, isOutput=False)
    gru_hh_rz = P("gru_hh_rz", [64, 128], F32R, isOutput=False)
    gru_ih_rz = P("gru_ih_rz", [1, 128], BF16, isOutput=False)
    gru_hh_n = P("gru_hh_n", [64, 64], F32R, isOutput=False)
    gru_ih_n = P("gru_ih_n", [1, 64], BF16, isOutput=False)
    bias_r = P("bias_r", [64, 1], F32, isOutput=False)
    bias_z = P("bias_z", [64, 1], F32, isOutput=False)
    bias_ihn = P("bias_ihn", [64, 1], F32, isOutput=False)
    bias_hhn = P("bias_hhn", [64, 1], F32, isOutput=False)
    wqT = P("wqT", [64, QK], F32R, isOutput=False)
    wkT = P("wkT", [64, QK], F32R, isOutput=False)
    wqb = P("wqb", [QK, 1], F32, isOutput=False)
    wkb = P("wkb", [QK, 1], F32, isOutput=False)
    ones64 = P("ones64", [64, 1], F32, isOutput=False)
    onesrow = P("onesrow", [1, SEQ], BF16, isOutput=False)
    bhhn_row = P("bhhn_row", [1, 64], BF16, isOutput=False)
    ident64f = P("ident64f", [64, 64], F32, isOutput=False)
    h0cp = P("h0cp", [128, CPF], BF16, isOutput=False)
    mwbd = P("mwbd", [15, 128, 128], BF16, isOutput=False)
    gbc0 = P("gbc0", [128, C * W], F32, isOutput=False)
    gbc1 = P("gbc1", [128, N * W], F32, isOutput=False)
    gbc2 = P("gbc2", [128, C * W], F32, isOutput=False)
    zerosb = P("zerosb", [128, 128], BF16, isOutput=False)
    lwT = P("lwT", [KCH, RO], BF16, isOutput=False)
    out = P("out", [B, RO], F32, isOutput=True)

    with tile.TileContext(nc) as tc:
        with tc.tile_pool(name="persist", bufs=1) as pp, \
             tc.tile_pool(name="finw", bufs=8) as fw, \
             tc.tile_pool(name="dram", bufs=1, space="DRAM") as dp:

            mw_sb = pp.tile([128, 15 * 128], BF16, tag="mw")
            mw_src = bass.AP(tensor=mwbd[:].tensor, offset=mwbd[:].offset,
                             ap=[[128, 128], [128 * 128, 15], [1, 128]])
            nc.sync.dma_start(mw_sb[:].rearrange("p (k m) -> p k m", k=15), mw_src)
            aggw = []
            for bpi in range(BP):
                t = pp.tile([128, 128], BF16, tag=f"aggw{bpi}")
                nc.sync.dma_start(t[:], zerosb[:])
                aggw.append(t)
            zb_sb = pp.tile([128, 128], BF16, tag="zb")
            nc.sync.dma_start(zb_sb[:], zerosb[:])
            scrA = dp.tile([128, BP * NWF], BF16, tag="scrA")
            scrB = dp.tile([128, BP * NWF], BF16, tag="scrB")
            scrC = dp.tile([128, BP * NWF], BF16, tag="scrC")
            a2a_in = dp.tile([B, KCH], BF16, tag="a2a_in")
            a2a_out = dp.tile([B, KCH], BF16, tag="a2a_out")
            ar_in = dp.tile([B, RO], F32, tag="ar_in")
            ar_out = dp.tile([B, RO], F32, tag="ar_out")

            # ================= GRU =================
            with tc.tile_pool(name="gru", bufs=1) as gp, \
                 tc.tile_pool(name="gwk", bufs=4) as gwk:

                x1_sb = gp.tile([1, W * SEQ], BF16, tag="x1")
                nc.sync.dma_start(x1_sb[:], x1[:])
                state = gp.tile([64, SEQ], F32R, tag="state")
                nc.sync.dma_start(state[:], state0[:])
                in_alls = []
                for ia in range(8):
                    in_t = gp.tile([64, 6 * SEQ], BF16, tag=f"in_all{ia}", name=f"in_all{ia}")
                    in_alls.append(in_t)

                hh_rz = gp.tile([64, 128], F32R, tag="hh_rz")
                nc.sync.dma_start(hh_rz[:], gru_hh_rz[:])
                ih_rz = gp.tile([1, 128], BF16, tag="ih_rz")
                nc.sync.dma_start(ih_rz[:], gru_ih_rz[:])
                hh_n = gp.tile([64, 64], F32R, tag="hh_n")
                nc.sync.dma_start(hh_n[:], gru_hh_n[:])
                ih_n = gp.tile([1, 64], BF16, tag="ih_n")
                nc.sync.dma_start(ih_n[:], gru_ih_n[:])
                b_r = gp.tile([64, 1], F32, tag="b_r")
                nc.sync.dma_start(b_r[:], bias_r[:])
                b_z = gp.tile([64, 1], F32, tag="b_z")
                nc.sync.dma_start(b_z[:], bias_z[:])
                b_ihn = gp.tile([64, 1], F32, tag="b_ihn")
                nc.sync.dma_start(b_ihn[:], bias_ihn[:])
                b_hhn = gp.tile([64, 1], F32, tag="b_hhn")
                nc.sync.dma_start(b_hhn[:], bias_hhn[:])
                ones_row = gp.tile([1, SEQ], BF16, tag="ones_row")
                nc.sync.dma_start(ones_row[:], onesrow[:])
                bhhn_r = gp.tile([1, 64], BF16, tag="bhhn_r")
                nc.sync.dma_start(bhhn_r[:], bhhn_row[:])

                with tc.tile_pool(name="gps", bufs=1, space="PSUM") as gps:
                    # in_ = w_ih_n * x + b_ihn for all steps (bf16)
                    for t in range(W):
                        pin = gps.tile([64, SEQ], F32, tag="pin")
                        nc.tensor.matmul(pin[:], ih_n[:],
                                         x1_sb[0:1, t * SEQ:(t + 1) * SEQ],
                                         start=True, stop=True)
                        nc.scalar.activation(
                            in_alls[t // 6][:, (t % 6) * SEQ:(t % 6 + 1) * SEQ],
                            pin[:], AF.Identity, bias=b_ihn[:])

                    CH = 256
                    for t in range(W):
                        for ch in range(2):
                            cs = ch * CH
                            prz = gps.tile([128, CH], F32, tag=f"prz{ch}")
                            nc.tensor.matmul(prz[:], hh_rz[:], state[:, cs:cs + CH],
                                             start=True, stop=False)
                            nc.tensor.matmul(prz[:], ih_rz[:],
                                             x1_sb[0:1, t * SEQ + cs: t * SEQ + cs + CH],
                                             start=False, stop=True)
                            pn = gps.tile([64, CH], F32, tag=f"pn{ch}")
                            nc.tensor.matmul(pn[:], hh_n[:], state[:, cs:cs + CH],
                                             start=True, stop=False)
                            nc.tensor.matmul(pn[:], bhhn_r[:], ones_row[0:1, 0:CH],
                                             start=False, stop=True)
                            rt = gwk.tile([64, CH], F32, tag=f"rt{ch}")
                            nc.scalar.activation(rt[:], prz[0:64, :], AF.Sigmoid, bias=b_r[:])
                            zt = gwk.tile([64, CH], F32, tag=f"zt{ch}")
                            nc.scalar.activation(zt[:], prz[64:128, :], AF.Sigmoid, bias=b_z[:])
                            t1 = gwk.tile([64, CH], F32, tag=f"t1{ch}")
                            nc.vector.tensor_mul(t1[:], rt[:], pn[:])
                            npre = gwk.tile([64, CH], F32, tag=f"npre{ch}")
                            nc.vector.tensor_add(
                                npre[:], t1[:],
                                in_alls[t // 6][:, (t % 6) * SEQ + cs: (t % 6) * SEQ + cs + CH])
                            nt = gwk.tile([64, CH], F32, tag=f"nt{ch}")
                            nc.scalar.activation(nt[:], npre[:], AF.Tanh)
                            dt_ = gwk.tile([64, CH], F32, tag=f"dt{ch}")
                            nc.vector.tensor_sub(dt_[:], state[0:64, cs:cs + CH].bitcast(F32), nt[:])
                            zd = gwk.tile([64, CH], F32, tag=f"zd{ch}")
                            nc.vector.tensor_mul(zd[:], zt[:], dt_[:])
                            nc.vector.tensor_add(state[0:64, cs:cs + CH], nt[:], zd[:])

                # ---- attention -> Anorm -> aggw quadrants ----
                wq_sb = gp.tile([64, QK], F32R, tag="wq")
                nc.sync.dma_start(wq_sb[:], wqT[:])
                wk_sb = gp.tile([64, QK], F32R, tag="wk")
                nc.sync.dma_start(wk_sb[:], wkT[:])
                wqb_sb = gp.tile([QK, 1], F32, tag="wqb")
                nc.sync.dma_start(wqb_sb[:], wqb[:])
                wkb_sb = gp.tile([QK, 1], F32, tag="wkb")
                nc.sync.dma_start(wkb_sb[:], wkb[:])
                ones_sb = gp.tile([64, 1], F32, tag="ones")
                nc.sync.dma_start(ones_sb[:], ones64[:])
                idf_sb = gp.tile([64, 64], F32, tag="idf")
                nc.sync.dma_start(idf_sb[:], ident64f[:])

                with tc.tile_pool(name="aps", bufs=1, space="PSUM") as aps:
                    pq = aps.tile([QK, SEQ], F32, tag="pq")
                    nc.tensor.matmul(pq[:], wq_sb[:], state[:], start=True, stop=True)
                    qt = gp.tile([QK, SEQ], F32, tag="qt")
                    nc.scalar.activation(qt[:], pq[:], AF.Identity, bias=wqb_sb[:])
                    pk = aps.tile([QK, SEQ], F32, tag="pk")
                    nc.tensor.matmul(pk[:], wk_sb[:], state[:], start=True, stop=True)
                    kt = gp.tile([QK, SEQ], F32, tag="kt")
                    nc.scalar.activation(kt[:], pk[:], AF.Identity, bias=wkb_sb[:])

                    for b in range(BL):
                        ps_ = aps.tile([64, 64], F32, tag="ps_s")
                        nc.tensor.matmul(ps_[:], qt[:, b * 64:(b + 1) * 64],
                                         kt[:, b * 64:(b + 1) * 64], start=True, stop=True)
                        s_sb = gwk.tile([64, 64], F32, tag="s_sb")
                        nc.scalar.activation(s_sb[:], ps_[:], AF.Identity,
                                             scale=1.0 / float(np.sqrt(QK)))
                        mx = gwk.tile([64, 1], F32, tag="mx")
                        nc.vector.tensor_reduce(out=mx[:], in_=s_sb[:], op=ALU.max,
                                                axis=mybir.AxisListType.X)
                        nmx = gwk.tile([64, 1], F32, tag="nmx")
                        nc.vector.tensor_scalar_mul(nmx[:], mx[:], -1.0)
                        ex = gwk.tile([64, 64], F32, tag="ex")
                        nc.scalar.activation(ex[:], s_sb[:], AF.Exp, bias=nmx[:])
                        sm = gwk.tile([64, 1], F32, tag="sm")
                        nc.vector.tensor_reduce(out=sm[:], in_=ex[:], op=ALU.add,
                                                axis=mybir.AxisListType.X)
                        rs = gwk.tile([64, 1], F32, tag="rs")
                        nc.vector.reciprocal(rs[:], sm[:])
                        a_sb = gwk.tile([64, 64], F32, tag="a_sb")
                        nc.vector.tensor_scalar_mul(a_sb[:], ex[:], rs[:])
                        pc = aps.tile([64, 1], F32, tag="pc")
                        nc.tensor.matmul(pc[:], a_sb[:], ones_sb[:], start=True, stop=True)
                        dsq = gwk.tile([64, 1], F32, tag="dsq")
                        nc.scalar.activation(dsq[:], pc[:], AF.Sqrt)
                        dinv = gwk.tile([64, 1], F32, tag="dinv")
                        nc.vector.reciprocal(dinv[:], dsq[:])
                        pr = aps.tile([1, 64], F32, tag="pr")
                        nc.tensor.matmul(pr[:], dinv[:], idf_sb[:], start=True, stop=True)
                        dinvr = gwk.tile([1, 64], F32, tag="dinvr")
                        nc.scalar.activation(dinvr[:], pr[:], AF.Identity)
                        po = aps.tile([64, 64], F32, tag="po")
                        nc.tensor.matmul(po[:], dinvr[:], dinvr[:], start=True, stop=True)
                        quad = aggw[b % BP][0:64, 0:64] if b < BP else aggw[b - BP][64:128, 64:128]
                        nc.vector.tensor_mul(quad, a_sb[:], po[:])

            # ================= blocks (conv0 | agg0 -> agg1 | conv1 -> conv2 | agg2) =================
            with tc.tile_pool(name="stg", bufs=2) as stg, \
                 tc.tile_pool(name="stg1", bufs=1) as stg1, \
                 tc.tile_pool(name="gbp", bufs=1) as gbp, \
                 tc.tile_pool(name="bwk", bufs=3) as bw, \
                 tc.tile_pool(name="bps", bufs=3, space="PSUM") as bps, \
                 tc.tile_pool(name="bps2", bufs=2, space="PSUM") as bps2:

                h0_sb = gbp.tile([128, CPF], BF16, tag="h0sb")
                nc.sync.dma_start(h0_sb[:], h0cp[:])
                gbc0_sb = gbp.tile([128, C * W], F32, tag="gbc0")
                nc.sync.dma_start(gbc0_sb[:], gbc0[:])
                gbc1_sb = gbp.tile([128, N * W], F32, tag="gbc1")
                nc.sync.dma_start(gbc1_sb[:], gbc1[:])
                gbc2_sb = gbp.tile([128, C * W], F32, tag="gbc2")
                nc.sync.dma_start(gbc2_sb[:], gbc2[:])
                for bl in range(BP):
                    # ---- conv0: CP -> CP (reads h0cp, merged gcn0+conv0, k=3) ----
                    hcpo = stg.tile([128, NWF], BF16, tag="hcpo0")
                    k = KS[0]
                    pad = k // 2
                    for wg in range(8):
                        base = bl * N * WP + 3 + wg * 6
                        p1 = bps.tile([128, N, 6], F32, tag="p1")
                        for t in range(k):
                            rhs = _ap(h0_sb[:], base + (t - pad), [[WP, N], [1, 6]])
                            nc.tensor.matmul(
                                p1[:], mw_sb[:, (TOFF[0] + t) * 128:(TOFF[0] + t + 1) * 128],
                                rhs, start=(t == 0), stop=(t == k - 1))
                        # evacuate to (n, w) flat tile
                        ldst = _ap(hcpo[:], wg * 6, [[W, N], [1, 6]])
                        nc.scalar.activation(ldst, p1[:], AF.Identity)
                    # ---- flip0: CP -> NP through scrA ----
                    for pl in range(2):
                        src = hcpo[pl * 64:pl * 64 + 64, :].rearrange("p (n w) -> p n w", n=N)
                        dst = _rap(scrA[:], pl * PLS + bl * NWF,
                                   [[W, 64], [BP * NWF, N], [1, W]])
                        nc.sync.dma_start(dst, src)
                    hnp0 = stg.tile([128, NWF], BF16, tag="hnp0")
                    for pl in range(2):
                        src = _rap(scrA[:], pl * PLS + bl * NWF,
                                   [[BP * NWF, 64], [1, NWF]])
                        nc.sync.dma_start(hnp0[pl * 64:pl * 64 + 64, :], src)
                    # ---- agg0 + gbc0 + lrelu: NP -> NP ----
                    h1np = stg.tile([128, NWF], BF16, tag="h1np")
                    for j in range(6):
                        pa = bps2.tile([128, 512], F32, tag="pa")
                        nc.tensor.matmul(pa[:], aggw[bl][:], hnp0[:, j * 512:(j + 1) * 512],
                                         start=True, stop=True)
                        s0 = bw.tile([128, 512], F32, tag="s0")
                        nc.vector.tensor_add(s0[:], pa[:], gbc0_sb[:, j * 512:(j + 1) * 512])
                        nc.scalar.activation(h1np[:, j * 512:(j + 1) * 512], s0[:], AF.Lrelu)
                    # ---- agg1: NP -> NP (no bias yet) ----
                    agnp = stg1.tile([128, NWF], BF16, tag="agnp")
                    for j in range(6):
                        pa = bps2.tile([128, 512], F32, tag="pa")
                        nc.tensor.matmul(pa[:], aggw[bl][:], h1np[:, j * 512:(j + 1) * 512],
                                         start=True, stop=True)
                        nc.scalar.activation(agnp[:, j * 512:(j + 1) * 512], pa[:], AF.Identity)
                    # ---- flip1: NP -> CP (padded) through scrB ----
                    for pl in range(2):
                        src = agnp[pl * 64:pl * 64 + 64, :].rearrange("p (c w) -> p c w", c=C)
                        dst = _rap(scrB[:], pl * PLS + bl * NWF,
                                   [[W, 64], [BP * NWF, C], [1, W]])
                        nc.sync.dma_start(dst, src)
                    hcp1 = stg1.tile([128, N * WP], BF16, tag="hcp1")
                    for off in (0, 3 + W):
                        zdst = _ap(hcp1[:], off, [[WP, N], [1, 3]])
                        zsrc = bass.AP(tensor=zb_sb[:].tensor, offset=zb_sb[:].offset,
                                       ap=[list(zb_sb[:].ap[0]), [0, N], [1, 3]])
                        nc.vector.tensor_copy(zdst, zsrc)
                    for pl in range(2):
                        src = _rap(scrB[:], pl * PLS + bl * NWF,
                                   [[BP * NWF, 64], [W, N], [1, W]])
                        dst = _ap(hcp1[pl * 64:pl * 64 + 64, :], 3, [[WP, N], [1, W]])
                        nc.sync.dma_start(dst, src)
                    # ---- conv1 + gbc1 + lrelu: CP -> CP padded (k=5) ----
                    h2cp = stg1.tile([128, N * WP], BF16, tag="h2cp")
                    for off in (0, 3 + W):
                        zdst = _ap(h2cp[:], off, [[WP, N], [1, 3]])
                        zsrc = bass.AP(tensor=zb_sb[:].tensor, offset=zb_sb[:].offset,
                                       ap=[list(zb_sb[:].ap[0]), [0, N], [1, 3]])
                        nc.vector.tensor_copy(zdst, zsrc)
                    k = KS[1]
                    pad = k // 2
                    for wg in range(8):
                        base = 3 + wg * 6
                        p1 = bps.tile([128, N, 6], F32, tag="p1")
                        for t in range(k):
                            rhs = _ap(hcp1[:], base + (t - pad), [[WP, N], [1, 6]])
                            nc.tensor.matmul(
                                p1[:], mw_sb[:, (TOFF[1] + t) * 128:(TOFF[1] + t + 1) * 128],
                                rhs, start=(t == 0), stop=(t == k - 1))
                        s1 = bw.tile([128, N, 6], F32, tag="s1")
                        gsl = _ap(gbc1_sb[:], wg * 6, [[W, N], [1, 6]])
                        nc.vector.tensor_add(s1[:], p1[:], gsl)
                        ldst = _ap(h2cp[:], 3 + wg * 6, [[WP, N], [1, 6]])
                        nc.scalar.activation(ldst, s1[:], AF.Lrelu)
                    # ---- conv2: CP -> CP (k=7) ----
                    hcpo2 = stg1.tile([128, NWF], BF16, tag="hcpo2")
                    k = KS[2]
                    pad = k // 2
                    for wg in range(8):
                        base = 3 + wg * 6
                        p1 = bps.tile([128, N, 6], F32, tag="p1")
                        for t in range(k):
                            rhs = _ap(h2cp[:], base + (t - pad), [[WP, N], [1, 6]])
                            nc.tensor.matmul(
                                p1[:], mw_sb[:, (TOFF[2] + t) * 128:(TOFF[2] + t + 1) * 128],
                                rhs, start=(t == 0), stop=(t == k - 1))
                        ldst = _ap(hcpo2[:], wg * 6, [[W, N], [1, 6]])
                        nc.scalar.activation(ldst, p1[:], AF.Identity)
                    # ---- flip2: CP -> NP through scrC ----
                    for pl in range(2):
                        src = hcpo2[pl * 64:pl * 64 + 64, :].rearrange("p (n w) -> p n w", n=N)
                        dst = _rap(scrC[:], pl * PLS + bl * NWF,
                                   [[W, 64], [BP * NWF, N], [1, W]])
                        nc.sync.dma_start(dst, src)
                    hnp2 = stg1.tile([128, NWF], BF16, tag="hnp2")
                    for pl in range(2):
                        src = _rap(scrC[:], pl * PLS + bl * NWF,
                                   [[BP * NWF, 64], [1, NWF]])
                        nc.sync.dma_start(hnp2[pl * 64:pl * 64 + 64, :], src)
                    # ---- agg2 + gbc2 + lrelu: NP -> (w,c) hst -> a2a send ----
                    hst = stg1.tile([128, C * W], BF16, tag="hst")
                    for j in range(8):
                        pa = bps2.tile([128, 8, W], F32, tag="pa2")
                        rhs = _ap(hnp2[:], j * 8 * W, [[W, 8], [1, W]])
                        nc.tensor.matmul(pa[:], aggw[bl][:], rhs, start=True, stop=True)
                        s2 = bw.tile([128, 8, W], F32, tag="s2")
                        gsl = _ap(gbc2_sb[:], j * 8 * W, [[W, 8], [1, W]])
                        nc.vector.tensor_add(s2[:], pa[:], gsl)
                        ldst = _ap(hst[:], j * 8, [[1, 8], [C, W]])
                        nc.scalar.activation(ldst, s2[:], AF.Lrelu)
                    for pl in range(2):
                        sl = hst[pl * 64:pl * 64 + 64, :]
                        for jj in range(8):
                            asrc = bass.AP(tensor=sl.tensor,
                                           offset=sl.offset + jj * 6 * C,
                                           ap=[list(sl.ap[0])] + [[C, 6], [1, C]])
                            adst = bass.AP(
                                tensor=a2a_in[:].tensor,
                                offset=a2a_in[:].offset + (jj * BL + pl * BP + bl) * KCH,
                                ap=[[C, N], [N * C, 6], [1, C]])
                            nc.sync.dma_start(adst, asrc)

            # ================= A2A + final linear + AR =================
            nc.gpsimd.collective_compute(
                "AllToAll", ALU.bypass,
                replica_groups=[list(range(NCORES))],
                ins=[a2a_in.opt()], outs=[a2a_out.opt()])

            with tc.tile_pool(name="fin", bufs=8) as fp, \
                 tc.tile_pool(name="fpo", bufs=1, space="PSUM") as fpo:

                pout = []
                for h in range(2):
                    po_t = fpo.tile([64, 384], F32, tag=f"pout{h}", name=f"pout{h}")
                    pout.append(po_t)
                NKT = KCH // 128
                for kt_ in range(NKT):
                    wt = fw.tile([128, RO], BF16, tag="wt")
                    weng = nc.sync if kt_ % 2 == 0 else nc.gpsimd
                    weng.dma_start(wt[:], lwT[kt_ * 128:(kt_ + 1) * 128, :])
                    ht = fp.tile([128, 64], BF16, tag="ht")
                    nc.scalar.dma_start(ht[:], a2a_out[:, kt_ * 128:(kt_ + 1) * 128],
                                        transpose=True)
                    for hh in range(2):
                        nc.tensor.matmul(pout[hh][:], ht[:], wt[:, hh * 384:(hh + 1) * 384],
                                         start=(kt_ == 0), stop=(kt_ == NKT - 1),
                                         skip_group_check=True)
                oo = fp.tile([64, RO], F32, tag="oo")
                for hh in range(2):
                    nc.scalar.activation(oo[:, hh * 384:(hh + 1) * 384], pout[hh][:], AF.Identity)
                nc.sync.dma_start(ar_in[:], oo[:])
                nc.gpsimd.collective_compute(
                    "AllReduce", ALU.add,
                    replica_groups=[list(range(NCORES))],
                    ins=[ar_in.opt()], outs=[ar_out.opt()])
                oo2 = fp.tile([64, RO], F32, tag="oo2")
                nc.sync.dma_start(oo2[:], ar_out[:])
                nc.sync.dma_start(out[:], oo2[:])

    nc.compile()
    return nc


_NC = None


def _host_prep(inputs):
    f32 = np.float32
    bf16 = ml_dtypes.bfloat16
    x = np.asarray(inputs["x"], f32)
    w_ih = np.asarray(inputs["gru_w_ih"], f32)
    w_hh = np.asarray(inputs["gru_w_hh"], f32)
    b_ih = np.asarray(inputs["gru_b_ih"], f32)
    b_hh = np.asarray(inputs["gru_b_hh"], f32)

    shared = {
        "state0": np.zeros((64, SEQ), f32),
        "gru_hh_rz": np.ascontiguousarray(w_hh[0:128].T),
        "gru_ih_rz": np.ascontiguousarray(w_ih[0:128, 0][None, :]).astype(bf16),
        "gru_hh_n": np.ascontiguousarray(w_hh[128:192].T),
        "gru_ih_n": np.ascontiguousarray(w_ih[128:192, 0][None, :]).astype(bf16),
        "bias_r": (b_ih + b_hh)[0:64, None].copy(),
        "bias_z": (b_ih + b_hh)[64:128, None].copy(),
        "bias_ihn": b_ih[128:192, None].copy(),
        "bias_hhn": b_hh[128:192, None].copy(),
        "wqT": np.ascontiguousarray(np.asarray(inputs["wq_w"], f32).T),
        "wkT": np.ascontiguousarray(np.asarray(inputs["wk_w"], f32).T),
        "wqb": np.asarray(inputs["wq_b"], f32)[:, None].copy(),
        "wkb": np.asarray(inputs["wk_b"], f32)[:, None].copy(),
        "ones64": np.ones((64, 1), f32),
        "onesrow": np.ones((1, SEQ)).astype(bf16),
        "bhhn_row": b_hh[128:192][None, :].astype(bf16),
        "ident64f": np.eye(64, dtype=f32),
        "zerosb": np.zeros((128, 128)).astype(bf16),
    }

    mwbd = np.zeros((15, 128, 128), f32)
    gbcs = []
    for i in range(3):
        gw_ = np.asarray(inputs[f"gcn_w{i}"], f32)
        gb = np.asarray(inputs[f"gcn_b{i}"], f32)
        cw = np.asarray(inputs[f"conv_w{i}"], f32)
        cb = np.asarray(inputs[f"conv_b{i}"], f32)
        k = KS[i]
        pad = k // 2
        for t in range(k):
            q = (cw[:, :, t] @ gw_).T         # lhsT quadrant [c_in, c_out]
            mwbd[TOFF[i] + t, 0:64, 0:64] = q
            mwbd[TOFF[i] + t, 64:128, 64:128] = q
        cgt = np.einsum("oit,i->ot", cw, gb)  # [o, k]
        g_ = np.zeros((C, W), f32)
        for w in range(W):
            for t in range(k):
                if 0 <= w + t - pad < W:
                    g_[:, w] += cgt[:, t]
        g_ += cb[:, None]
        gbcs.append(g_)

    shared["mwbd"] = mwbd.astype(bf16)
    # gbc0/gbc2: NP layout, free (c, w), broadcast over (pl, n) partitions
    shared["gbc0"] = np.tile(gbcs[0].reshape(C * W), (128, 1))
    shared["gbc2"] = np.tile(gbcs[2].reshape(C * W), (128, 1))
    # gbc1: CP layout, free (n, w), partition (pl, c_out)
    m1 = np.tile(gbcs[1][:, None, :], (1, N, 1)).reshape(C, N * W)
    shared["gbc1"] = np.vstack([m1, m1])

    emb_w = np.asarray(inputs["emb_w"], f32)
    emb_b = np.asarray(inputs["emb_b"], f32)
    lout_w = np.asarray(inputs["lout_w"], f32)

    in_maps = []
    for c_ in range(NCORES):
        xc = x[c_ * BL:(c_ + 1) * BL]
        m = dict(shared)
        m["x1"] = np.ascontiguousarray(
            xc.transpose(1, 0, 2).reshape(1, W * SEQ)).astype(bf16)
        h0 = xc[..., None] * emb_w + emb_b                  # [8, 48, 64, 64]
        hcp_h = np.zeros((2, 64, BP, N, WP), f32)
        hsrc = h0.reshape(2, BP, W, N, C).transpose(0, 4, 1, 3, 2)
        hcp_h[:, :, :, :, 3:3 + W] = hsrc
        m["h0cp"] = np.ascontiguousarray(hcp_h.reshape(128, CPF)).astype(bf16)
        lw = lout_w[:, c_ * KCH:(c_ + 1) * KCH]
        m["lwT"] = np.ascontiguousarray(lw.T).astype(bf16)
        in_maps.append(m)
    return in_maps


def kernel_with_stats(**inputs):
    global _NC
    if _NC is None:
        _NC = _build()
    in_maps = _host_prep(inputs)
    trace = os.environ.get("KERNEL_TRACE", "") == "1"
    res = run_bass_kernel_spmd(_NC, in_maps, core_ids=list(range(NCORES)), trace=trace)
    out = res.results[0]["out"] + np.asarray(inputs["lout_b"], np.float32)[None, :]
    return out.reshape(B, HOR, N).astype(np.float32), res


def kernel(**inputs):
    o, _ = kernel_with_stats(**inputs)
    return o


# revision 8
# speedup vs baseline: 1.2241x; 1.2241x over previous
"""Trainium2 Bass kernel for BaselineBlockNetSingleGraph (GRU + attention-GCN + convs + big linear).

Sharding: data-parallel over batch B=64 across 8 cores (8 batches/core) for
everything up to the final linear; the final linear's 196608-wide reduction is
column-sharded across cores (24576 each) via an on-device AllToAll of the
activations (bf16), with an AllReduce of the [64, 768] partials.

v2 restructure: since the GCN aggregation (n-axis), merged gcn+conv (c,w axes)
are commuting linear maps, per-block order alternates so each block needs only
ONE layout flip, done by strided DMA through DRAM (no PE transposes):
  block0: conv0 (CP) -> flip -> agg0+bias+lrelu (NP)
  block1: agg1 (NP) -> flip -> conv1+bias+lrelu (CP, padded)
  block2: conv2 (CP) -> flip -> agg2+bias+lrelu (NP) -> A2A send
All conv/agg matmuls in bf16. Final linear uses DMA XBAR transposes for the
[64,k]->[k,64] activation tiles instead of PE transposes.

On-chip layouts (per core; plane = batch half, plane0 = local batches 0-3):
  CP: [128 = (plane, c), (n=64, wp=54)] per local batch (w padded by 3 both sides)
  NP: [128 = (plane, n), (c=64, w=48)] per local batch
"""

import os
import numpy as np
import ml_dtypes

import concourse.bass as bass
import concourse.tile as tile
from concourse import mybir, bacc
from concourse.bass_utils import run_bass_kernel_spmd

F32 = mybir.dt.float32
F32R = mybir.dt.float32r
BF16 = mybir.dt.bfloat16
AF = mybir.ActivationFunctionType
ALU = mybir.AluOpType

B, W, N, C, H, QK, HOR = 64, 48, 64, 64, 64, 32, 12
NCORES = 8
BL = B // NCORES          # 8 local batches
BP = BL // 2              # 4 batches per plane
SEQ = BL * N              # 512 sequences per core
WP = W + 6                # padded w
KCH = W * N * C // NCORES # 24576 reduction chunk per core
KS = [3, 5, 7]
TOFF = [0, 3, 8]
CPF = BP * N * WP         # 13824
RO = N * HOR              # 768
NWF = N * W               # 3072 per-batch flat (n,w) or (c,w)
PLS = 64 * BP * NWF       # 786432: plane stride in flip scratch


def _ap(base_ap, off, dims):
    """AP with same partition dim as base_ap, extra element offset, given free dims."""
    return bass.AP(tensor=base_ap.tensor, offset=base_ap.offset + off,
                   ap=[list(base_ap.ap[0])] + [list(d) for d in dims])


def _rap(base_ap, off, dims):
    """Raw AP on a DRAM tile: explicit dims incl. the partition-paired one."""
    return bass.AP(tensor=base_ap.tensor, offset=base_ap.offset + off,
                   ap=[list(d) for d in dims])


def _build():
    nc = bacc.Bacc("TRN2", target_bir_lowering=False, debug=False, num_devices=NCORES)
    P = nc.declare_dram_parameter

    x1 = P("x1", [1, W * SEQ], BF16, isOutput=False)
    state0 = P("state0", [64, SEQ], F32R, isOutput=False)
    gru_hh_rz = P("gru_hh_rz", [64, 128], F32R, isOutput=False)
    gru_ih_rz = P("gru_ih_rz", [1, 128], BF16, isOutput=False)
    gru_hh_n = P("gru_hh_n", [64, 64], F32R, isOutput=False)
    gru_ih_n = P("gru_ih_n", [1, 64], BF16, isOutput=False)
    bias_r = P("bias_r", [64, 1], F32, isOutput=False)
    bias_z = P("bias_z", [64, 1], F32, isOutput=False)
    bias_ihn = P("bias_ihn", [64, 1], F32, isOutput=False)
    bias_hhn = P("bias_hhn", [64, 1], F32, isOutput=False)
    wqT = P("wqT", [64, QK], F32R, isOutput=False)
    wkT = P("wkT", [64, QK], F32R, isOutput=False)
    wqb = P("wqb", [QK, 1], F32, isOutput=False)
    wkb = P("wkb", [QK, 1], F32, isOutput=False)
    ones64 = P("ones64", [64, 1], F32, isOutput=False)
    onesrow = P("onesrow", [1, SEQ], BF16, isOutput=False)
    bhhn_row = P("bhhn_row", [1, 64], BF16, isOutput=False)
    ident64f = P("ident64f", [64, 64], F32, isOutput=False)
    h0cp = P("h0cp", [128, CPF], BF16, isOutput=False)
    mwbd = P("mwbd", [15, 128, 128], BF16, isOutput=False)
    gbc0r = P("gbc0r", [1, C * W], BF16, isOutput=False)
    gbc1f = P("gbc1f", [128, N * W], F32, isOutput=False)
    gbc2r = P("gbc2r", [1, C * W], BF16, isOutput=False)
    identb = P("identb", [64, 64], BF16, isOutput=False)
    zerosb = P("zerosb", [128, 128], BF16, isOutput=False)
    lwT = P("lwT", [KCH, RO], BF16, isOutput=False)
    out = P("out", [B, RO], F32, isOutput=True)

    with tile.TileContext(nc) as tc:
        with tc.tile_pool(name="persist", bufs=1) as pp, \
             tc.tile_pool(name="finw", bufs=8) as fw, \
             tc.tile_pool(name="dram", bufs=1, space="DRAM") as dp:

            mw_sb = pp.tile([128, 15 * 128], BF16, tag="mw")
            mw_src = bass.AP(tensor=mwbd[:].tensor, offset=mwbd[:].offset,
                             ap=[[128, 128], [128 * 128, 15], [1, 128]])
            nc.sync.dma_start(mw_sb[:].rearrange("p (k m) -> p k m", k=15), mw_src)
            aggw = []
            for bpi in range(BP):
                t = pp.tile([128, 128], BF16, tag=f"aggw{bpi}")
                nc.sync.dma_start(t[:], zerosb[:])
                aggw.append(t)
            zb_sb = pp.tile([128, 128], BF16, tag="zb")
            nc.sync.dma_start(zb_sb[:], zerosb[:])
            id_sb = pp.tile([64, 64], BF16, tag="ident")
            nc.sync.dma_start(id_sb[:], identb[:])
            ones_row = pp.tile([1, SEQ], BF16, tag="ones_row")
            nc.sync.dma_start(ones_row[:], onesrow[:])
            h0_sb = pp.tile([128, CPF], BF16, tag="h0sb")
            nc.scalar.dma_start(h0_sb[:], h0cp[:])
            scrA = dp.tile([128, BP * NWF], BF16, tag="scrA")
            scrB = dp.tile([128, BP * NWF], BF16, tag="scrB")
            scrC = dp.tile([128, BP * NWF], BF16, tag="scrC")
            a2a_in = dp.tile([B, KCH], BF16, tag="a2a_in")
            a2a_out = dp.tile([B, KCH], BF16, tag="a2a_out")
            ar_in = dp.tile([B, RO], F32, tag="ar_in")
            ar_out = dp.tile([B, RO], F32, tag="ar_out")

            # ================= GRU =================
            with tc.tile_pool(name="gru", bufs=1) as gp, \
                 tc.tile_pool(name="gwk", bufs=3) as gwk:

                x1_sb = gp.tile([1, W * SEQ], BF16, tag="x1")
                nc.sync.dma_start(x1_sb[:], x1[:])
                state = gp.tile([64, SEQ], F32R, tag="state")
                nc.sync.dma_start(state[:], state0[:])
                in_alls = []
                for ia in range(8):
                    in_t = gp.tile([64, 6 * SEQ], BF16, tag=f"in_all{ia}", name=f"in_all{ia}")
                    in_alls.append(in_t)

                hh_rz = gp.tile([64, 128], F32R, tag="hh_rz")
                nc.sync.dma_start(hh_rz[:], gru_hh_rz[:])
                ih_rz = gp.tile([1, 128], BF16, tag="ih_rz")
                nc.sync.dma_start(ih_rz[:], gru_ih_rz[:])
                hh_n = gp.tile([64, 64], F32R, tag="hh_n")
                nc.sync.dma_start(hh_n[:], gru_hh_n[:])
                ih_n = gp.tile([1, 64], BF16, tag="ih_n")
                nc.sync.dma_start(ih_n[:], gru_ih_n[:])
                b_r = gp.tile([64, 1], F32, tag="b_r")
                nc.sync.dma_start(b_r[:], bias_r[:])
                b_z = gp.tile([64, 1], F32, tag="b_z")
                nc.sync.dma_start(b_z[:], bias_z[:])
                b_ihn = gp.tile([64, 1], F32, tag="b_ihn")
                nc.sync.dma_start(b_ihn[:], bias_ihn[:])
                b_hhn = gp.tile([64, 1], F32, tag="b_hhn")
                nc.sync.dma_start(b_hhn[:], bias_hhn[:])
                bhhn_r = gp.tile([1, 64], BF16, tag="bhhn_r")
                nc.sync.dma_start(bhhn_r[:], bhhn_row[:])

                with tc.tile_pool(name="gps", bufs=1, space="PSUM") as gps:
                    # in_ = w_ih_n * x + b_ihn for all steps (bf16)
                    for t in range(W):
                        pin = gps.tile([64, SEQ], F32, tag="pin")
                        nc.tensor.matmul(pin[:], ih_n[:],
                                         x1_sb[0:1, t * SEQ:(t + 1) * SEQ],
                                         start=True, stop=True)
                        nc.scalar.activation(
                            in_alls[t // 6][:, (t % 6) * SEQ:(t % 6 + 1) * SEQ],
                            pin[:], AF.Identity, bias=b_ihn[:])

                    CH = 256
                    for t in range(W):
                        for ch in range(2):
                            cs = ch * CH
                            prz = gps.tile([128, CH], F32, tag=f"prz{ch}")
                            nc.tensor.matmul(prz[:], hh_rz[:], state[:, cs:cs + CH],
                                             start=True, stop=False)
                            nc.tensor.matmul(prz[:], ih_rz[:],
                                             x1_sb[0:1, t * SEQ + cs: t * SEQ + cs + CH],
                                             start=False, stop=True)
                            pn = gps.tile([64, CH], F32, tag=f"pn{ch}")
                            nc.tensor.matmul(pn[:], hh_n[:], state[:, cs:cs + CH],
                                             start=True, stop=False)
                            nc.tensor.matmul(pn[:], bhhn_r[:], ones_row[0:1, 0:CH],
                                             start=False, stop=True)
                            rt = gwk.tile([64, CH], F32, tag=f"rt{ch}")
                            nc.scalar.activation(rt[:], prz[0:64, :], AF.Sigmoid, bias=b_r[:])
                            zt = gwk.tile([64, CH], F32, tag=f"zt{ch}")
                            nc.scalar.activation(zt[:], prz[64:128, :], AF.Sigmoid, bias=b_z[:])
                            t1 = gwk.tile([64, CH], F32, tag=f"t1{ch}")
                            nc.vector.tensor_mul(t1[:], rt[:], pn[:])
                            npre = gwk.tile([64, CH], F32, tag=f"npre{ch}")
                            nc.vector.tensor_add(
                                npre[:], t1[:],
                                in_alls[t // 6][:, (t % 6) * SEQ + cs: (t % 6) * SEQ + cs + CH])
                            nt = gwk.tile([64, CH], F32, tag=f"nt{ch}")
                            nc.scalar.activation(nt[:], npre[:], AF.Tanh)
                            dt_ = gwk.tile([64, CH], F32, tag=f"dt{ch}")
                            nc.vector.tensor_sub(dt_[:], state[0:64, cs:cs + CH].bitcast(F32), nt[:])
                            zd = gwk.tile([64, CH], F32, tag=f"zd{ch}")
                            nc.vector.tensor_mul(zd[:], zt[:], dt_[:])
                            nc.vector.tensor_add(state[0:64, cs:cs + CH], nt[:], zd[:])

                # ---- attention -> Anorm -> aggw quadrants ----
                wq_sb = gp.tile([64, QK], F32R, tag="wq")
                nc.sync.dma_start(wq_sb[:], wqT[:])
                wk_sb = gp.tile([64, QK], F32R, tag="wk")
                nc.sync.dma_start(wk_sb[:], wkT[:])
                wqb_sb = gp.tile([QK, 1], F32, tag="wqb")
                nc.sync.dma_start(wqb_sb[:], wqb[:])
                wkb_sb = gp.tile([QK, 1], F32, tag="wkb")
                nc.sync.dma_start(wkb_sb[:], wkb[:])
                ones_sb = gp.tile([64, 1], F32, tag="ones")
                nc.sync.dma_start(ones_sb[:], ones64[:])
                idf_sb = gp.tile([64, 64], F32, tag="idf")
                nc.sync.dma_start(idf_sb[:], ident64f[:])

                with tc.tile_pool(name="aps", bufs=1, space="PSUM") as aps:
                    pq = aps.tile([QK, SEQ], F32, tag="pq")
                    nc.tensor.matmul(pq[:], wq_sb[:], state[:], start=True, stop=True)
                    qt = gp.tile([QK, SEQ], F32, tag="qt")
                    nc.scalar.activation(qt[:], pq[:], AF.Identity, bias=wqb_sb[:])
                    pk = aps.tile([QK, SEQ], F32, tag="pk")
                    nc.tensor.matmul(pk[:], wk_sb[:], state[:], start=True, stop=True)
                    kt = gp.tile([QK, SEQ], F32, tag="kt")
                    nc.scalar.activation(kt[:], pk[:], AF.Identity, bias=wkb_sb[:])

                    for b in range(BL):
                        ps_ = aps.tile([64, 64], F32, tag="ps_s")
                        nc.tensor.matmul(ps_[:], qt[:, b * 64:(b + 1) * 64],
                                         kt[:, b * 64:(b + 1) * 64], start=True, stop=True)
                        s_sb = gwk.tile([64, 64], F32, tag="s_sb")
                        nc.scalar.activation(s_sb[:], ps_[:], AF.Identity,
                                             scale=1.0 / float(np.sqrt(QK)))
                        mx = gwk.tile([64, 1], F32, tag="mx")
                        nc.vector.tensor_reduce(out=mx[:], in_=s_sb[:], op=ALU.max,
                                                axis=mybir.AxisListType.X)
                        nmx = gwk.tile([64, 1], F32, tag="nmx")
                        nc.vector.tensor_scalar_mul(nmx[:], mx[:], -1.0)
                        ex = gwk.tile([64, 64], F32, tag="ex")
                        nc.scalar.activation(ex[:], s_sb[:], AF.Exp, bias=nmx[:])
                        sm = gwk.tile([64, 1], F32, tag="sm")
                        nc.vector.tensor_reduce(out=sm[:], in_=ex[:], op=ALU.add,
                                                axis=mybir.AxisListType.X)
                        rs = gwk.tile([64, 1], F32, tag="rs")
                        nc.vector.reciprocal(rs[:], sm[:])
                        a_sb = gwk.tile([64, 64], F32, tag="a_sb")
                        nc.vector.tensor_scalar_mul(a_sb[:], ex[:], rs[:])
                        pc = aps.tile([64, 1], F32, tag="pc")
                        nc.tensor.matmul(pc[:], a_sb[:], ones_sb[:], start=True, stop=True)
                        dsq = gwk.tile([64, 1], F32, tag="dsq")
                        nc.scalar.activation(dsq[:], pc[:], AF.Sqrt)
                        dinv = gwk.tile([64, 1], F32, tag="dinv")
                        nc.vector.reciprocal(dinv[:], dsq[:])
                        pr = aps.tile([1, 64], F32, tag="pr")
                        nc.tensor.matmul(pr[:], dinv[:], idf_sb[:], start=True, stop=True)
                        dinvr = gwk.tile([1, 64], F32, tag="dinvr")
                        nc.scalar.activation(dinvr[:], pr[:], AF.Identity)
                        po = aps.tile([64, 64], F32, tag="po")
                        nc.tensor.matmul(po[:], dinvr[:], dinvr[:], start=True, stop=True)
                        quad = aggw[b % BP][0:64, 0:64] if b < BP else aggw[b - BP][64:128, 64:128]
                        nc.vector.tensor_mul(quad, a_sb[:], po[:])

            # ================= blocks (conv0 | agg0 -> agg1 | conv1 -> conv2 | agg2) =================
            with tc.tile_pool(name="stg", bufs=2) as stg, \
                 tc.tile_pool(name="stg1", bufs=2) as stg1, \
                 tc.tile_pool(name="gbp", bufs=1) as gbp, \
                 tc.tile_pool(name="bwk", bufs=3) as bw, \
                 tc.tile_pool(name="bps", bufs=3, space="PSUM") as bps, \
                 tc.tile_pool(name="bps2", bufs=2, space="PSUM") as bps2:

                gbc0_sb = gbp.tile([1, C * W], BF16, tag="gbc0")
                nc.gpsimd.dma_start(gbc0_sb[:], gbc0r[:])
                gbc1_sb = gbp.tile([128, N * W], F32, tag="gbc1")
                nc.gpsimd.dma_start(gbc1_sb[:], gbc1f[:])
                gbc2_sb = gbp.tile([1, C * W], BF16, tag="gbc2")
                nc.gpsimd.dma_start(gbc2_sb[:], gbc2r[:])
                for bl in range(BP):
                    # ---- conv0: CP -> CP (reads h0cp, merged gcn0+conv0, k=3) ----
                    hcpo = stg.tile([128, NWF], BF16, tag="hcpo0")
                    k = KS[0]
                    pad = k // 2
                    for wg in range(8):
                        base = bl * N * WP + 3 + wg * 6
                        p1 = bps.tile([128, N, 6], F32, tag="p1")
                        for t in range(k):
                            rhs = _ap(h0_sb[:], base + (t - pad), [[WP, N], [1, 6]])
                            nc.tensor.matmul(
                                p1[:], mw_sb[:, (TOFF[0] + t) * 128:(TOFF[0] + t + 1) * 128],
                                rhs, start=(t == 0), stop=(t == k - 1))
                        # evacuate to (n, w) flat tile
                        ldst = _ap(hcpo[:], wg * 6, [[W, N], [1, 6]])
                        nc.scalar.activation(ldst, p1[:], AF.Identity)
                    # ---- flip0: CP -> NP through scrA ----
                    for pl in range(2):
                        src = hcpo[pl * 64:pl * 64 + 64, :].rearrange("p (n w) -> p n w", n=N)
                        dst = _rap(scrA[:], pl * PLS + bl * NWF,
                                   [[W, 64], [BP * NWF, N], [1, W]])
                        nc.sync.dma_start(dst, src)
                    hnp0 = stg.tile([128, NWF], BF16, tag="hnp0")
                    for pl in range(2):
                        src = _rap(scrA[:], pl * PLS + bl * NWF,
                                   [[BP * NWF, 64], [1, NWF]])
                        nc.gpsimd.dma_start(hnp0[pl * 64:pl * 64 + 64, :], src)
                    # ---- agg0 + gbc0 + lrelu: NP -> NP ----
                    h1np = stg.tile([128, NWF], BF16, tag="h1np")
                    for j in range(6):
                        pa = bps2.tile([128, 512], F32, tag="pa")
                        nc.tensor.matmul(pa[:], aggw[bl][:], hnp0[:, j * 512:(j + 1) * 512],
                                         start=True, stop=False)
                        nc.tensor.matmul(pa[:], ones_row[0:1, 0:128],
                                         gbc0_sb[0:1, j * 512:(j + 1) * 512],
                                         start=False, stop=True)
                        nc.scalar.activation(h1np[:, j * 512:(j + 1) * 512], pa[:], AF.Lrelu)
                    # ---- agg1: NP -> NP (no bias yet) ----
                    agnp = stg1.tile([128, NWF], BF16, tag="agnp")
                    for j in range(6):
                        pa = bps2.tile([128, 512], F32, tag="pa")
                        nc.tensor.matmul(pa[:], aggw[bl][:], h1np[:, j * 512:(j + 1) * 512],
                                         start=True, stop=True)
                        nc.scalar.activation(agnp[:, j * 512:(j + 1) * 512], pa[:], AF.Identity)
                    # ---- flip1: NP -> CP (padded) through scrB ----
                    for pl in range(2):
                        src = agnp[pl * 64:pl * 64 + 64, :].rearrange("p (c w) -> p c w", c=C)
                        dst = _rap(scrB[:], pl * PLS + bl * NWF,
                                   [[W, 64], [BP * NWF, C], [1, W]])
                        nc.sync.dma_start(dst, src)
                    hcp1 = stg1.tile([128, N * WP], BF16, tag="hcp1")
                    for off in (0, 3 + W):
                        zdst = _ap(hcp1[:], off, [[WP, N], [1, 3]])
                        zsrc = bass.AP(tensor=zb_sb[:].tensor, offset=zb_sb[:].offset,
                                       ap=[list(zb_sb[:].ap[0]), [0, N], [1, 3]])
                        nc.vector.tensor_copy(zdst, zsrc)
                    for pl in range(2):
                        src = _rap(scrB[:], pl * PLS + bl * NWF,
                                   [[BP * NWF, 64], [W, N], [1, W]])
                        dst = _ap(hcp1[pl * 64:pl * 64 + 64, :], 3, [[WP, N], [1, W]])
                        nc.gpsimd.dma_start(dst, src)
                    # ---- conv1 + gbc1 + lrelu: CP -> CP padded (k=5) ----
                    h2cp = stg1.tile([128, N * WP], BF16, tag="h2cp")
                    for off in (0, 3 + W):
                        zdst = _ap(h2cp[:], off, [[WP, N], [1, 3]])
                        zsrc = bass.AP(tensor=zb_sb[:].tensor, offset=zb_sb[:].offset,
                                       ap=[list(zb_sb[:].ap[0]), [0, N], [1, 3]])
                        nc.vector.tensor_copy(zdst, zsrc)
                    k = KS[1]
                    pad = k // 2
                    for wg in range(8):
                        base = 3 + wg * 6
                        p1 = bps.tile([128, N, 6], F32, tag="p1")
                        for t in range(k):
                            rhs = _ap(hcp1[:], base + (t - pad), [[WP, N], [1, 6]])
                            nc.tensor.matmul(
                                p1[:], mw_sb[:, (TOFF[1] + t) * 128:(TOFF[1] + t + 1) * 128],
                                rhs, start=(t == 0), stop=(t == k - 1))
                        s1 = bw.tile([128, N, 6], F32, tag="s1")
                        gsl = _ap(gbc1_sb[:], wg * 6, [[W, N], [1, 6]])
                        nc.vector.tensor_add(s1[:], p1[:], gsl)
                        ldst = _ap(h2cp[:], 3 + wg * 6, [[WP, N], [1, 6]])
                        nc.scalar.activation(ldst, s1[:], AF.Lrelu)
                    # ---- conv2: CP -> CP (k=7) ----
                    hcpo2 = stg1.tile([128, NWF], BF16, tag="hcpo2")
                    k = KS[2]
                    pad = k // 2
                    for wg in range(8):
                        base = 3 + wg * 6
                        p1 = bps.tile([128, N, 6], F32, tag="p1")
                        for t in range(k):
                            rhs = _ap(h2cp[:], base + (t - pad), [[WP, N], [1, 6]])
                            nc.tensor.matmul(
                                p1[:], mw_sb[:, (TOFF[2] + t) * 128:(TOFF[2] + t + 1) * 128],
                                rhs, start=(t == 0), stop=(t == k - 1))
                        ldst = _ap(hcpo2[:], wg * 6, [[W, N], [1, 6]])
                        nc.scalar.activation(ldst, p1[:], AF.Identity)
                    # ---- flip2: CP -> NP through scrC ----
                    for pl in range(2):
                        src = hcpo2[pl * 64:pl * 64 + 64, :].rearrange("p (n w) -> p n w", n=N)
                        dst = _rap(scrC[:], pl * PLS + bl * NWF,
                                   [[W, 64], [BP * NWF, N], [1, W]])
                        nc.sync.dma_start(dst, src)
                    hnp2 = stg1.tile([128, NWF], BF16, tag="hnp2")
                    for pl in range(2):
                        src = _rap(scrC[:], pl * PLS + bl * NWF,
                                   [[BP * NWF, 64], [1, NWF]])
                        nc.gpsimd.dma_start(hnp2[pl * 64:pl * 64 + 64, :], src)
                    # ---- agg2 + gbc2 + lrelu: NP -> (w,c) hst -> a2a send ----
                    hst = stg1.tile([128, C * W], BF16, tag="hst")
                    for j in range(8):
                        pa = bps2.tile([128, W, 8], F32, tag="pa2")
                        rhs = _ap(hnp2[:], j * 8 * W, [[1, W], [W, 8]])
                        nc.tensor.matmul(pa[:], aggw[bl][:], rhs, start=True, stop=False)
                        gsl = bass.AP(tensor=gbc2_sb[:].tensor,
                                      offset=gbc2_sb[:].offset + j * 8 * W,
                                      ap=[list(gbc2_sb[:].ap[0]), [1, W], [W, 8]])
                        nc.tensor.matmul(pa[:], ones_row[0:1, 0:128], gsl,
                                         start=False, stop=True)
                        ldst = _ap(hst[:], j * 8, [[C, W], [1, 8]])
                        nc.scalar.activation(ldst, pa[:], AF.Lrelu)
                    for pl in range(2):
                        sl = hst[pl * 64:pl * 64 + 64, :]
                        for jj in range(8):
                            asrc = bass.AP(tensor=sl.tensor,
                                           offset=sl.offset + jj * 6 * C,
                                           ap=[list(sl.ap[0])] + [[C, 6], [1, C]])
                            adst = bass.AP(
                                tensor=a2a_in[:].tensor,
                                offset=a2a_in[:].offset + (jj * BL + pl * BP + bl) * KCH,
                                ap=[[C, N], [N * C, 6], [1, C]])
                            nc.sync.dma_start(adst, asrc)

            # ================= A2A + final linear + AR =================
            nc.gpsimd.collective_compute(
                "AllToAll", ALU.bypass,
                replica_groups=[list(range(NCORES))],
                ins=[a2a_in.opt()], outs=[a2a_out.opt()])

            with tc.tile_pool(name="fin", bufs=8) as fp, \
                 tc.tile_pool(name="fps", bufs=3, space="PSUM") as fps, \
                 tc.tile_pool(name="fpo", bufs=1, space="PSUM") as fpo:

                pout = []
                for h in range(2):
                    po_t = fpo.tile([64, 384], F32, tag=f"pout{h}", name=f"pout{h}")
                    pout.append(po_t)
                NKT = KCH // 128
                for kt_ in range(NKT):
                    wt = fw.tile([128, RO], BF16, tag="wt")
                    weng = nc.sync if kt_ % 2 == 0 else nc.gpsimd
                    weng.dma_start(wt[:], lwT[kt_ * 128:(kt_ + 1) * 128, :])
                    ho = fp.tile([64, 128], BF16, tag="ho")
                    nc.scalar.dma_start(ho[:], a2a_out[:, kt_ * 128:(kt_ + 1) * 128])
                    pt = fps.tile([128, 64], BF16, tag="pt")
                    nc.tensor.transpose(pt[:], ho[:], id_sb[:])
                    ht = fp.tile([128, 64], BF16, tag="ht")
                    nc.scalar.activation(ht[:], pt[:], AF.Identity)
                    for hh in range(2):
                        nc.tensor.matmul(pout[hh][:], ht[:], wt[:, hh * 384:(hh + 1) * 384],
                                         start=(kt_ == 0), stop=(kt_ == NKT - 1),
                                         skip_group_check=True)
                oo = fp.tile([64, RO], F32, tag="oo")
                for hh in range(2):
                    nc.scalar.activation(oo[:, hh * 384:(hh + 1) * 384], pout[hh][:], AF.Identity)
                nc.sync.dma_start(ar_in[:], oo[:])
                nc.gpsimd.collective_compute(
                    "AllReduce", ALU.add,
                    replica_groups=[list(range(NCORES))],
                    ins=[ar_in.opt()], outs=[ar_out.opt()])
                oo2 = fp.tile([64, RO], F32, tag="oo2")
                nc.sync.dma_start(oo2[:], ar_out[:])
                nc.sync.dma_start(out[:], oo2[:])

    nc.compile()
    return nc


_NC = None


def _host_prep(inputs):
    f32 = np.float32
    bf16 = ml_dtypes.bfloat16
    x = np.asarray(inputs["x"], f32)
    w_ih = np.asarray(inputs["gru_w_ih"], f32)
    w_hh = np.asarray(inputs["gru_w_hh"], f32)
    b_ih = np.asarray(inputs["gru_b_ih"], f32)
    b_hh = np.asarray(inputs["gru_b_hh"], f32)

    shared = {
        "state0": np.zeros((64, SEQ), f32),
        "gru_hh_rz": np.ascontiguousarray(w_hh[0:128].T),
        "gru_ih_rz": np.ascontiguousarray(w_ih[0:128, 0][None, :]).astype(bf16),
        "gru_hh_n": np.ascontiguousarray(w_hh[128:192].T),
        "gru_ih_n": np.ascontiguousarray(w_ih[128:192, 0][None, :]).astype(bf16),
        "bias_r": (b_ih + b_hh)[0:64, None].copy(),
        "bias_z": (b_ih + b_hh)[64:128, None].copy(),
        "bias_ihn": b_ih[128:192, None].copy(),
        "bias_hhn": b_hh[128:192, None].copy(),
        "wqT": np.ascontiguousarray(np.asarray(inputs["wq_w"], f32).T),
        "wkT": np.ascontiguousarray(np.asarray(inputs["wk_w"], f32).T),
        "wqb": np.asarray(inputs["wq_b"], f32)[:, None].copy(),
        "wkb": np.asarray(inputs["wk_b"], f32)[:, None].copy(),
        "ones64": np.ones((64, 1), f32),
        "onesrow": np.ones((1, SEQ)).astype(bf16),
        "bhhn_row": b_hh[128:192][None, :].astype(bf16),
        "ident64f": np.eye(64, dtype=f32),
        "zerosb": np.zeros((128, 128)).astype(bf16),
    }

    mwbd = np.zeros((15, 128, 128), f32)
    gbcs = []
    for i in range(3):
        gw_ = np.asarray(inputs[f"gcn_w{i}"], f32)
        gb = np.asarray(inputs[f"gcn_b{i}"], f32)
        cw = np.asarray(inputs[f"conv_w{i}"], f32)
        cb = np.asarray(inputs[f"conv_b{i}"], f32)
        k = KS[i]
        pad = k // 2
        for t in range(k):
            q = (cw[:, :, t] @ gw_).T         # lhsT quadrant [c_in, c_out]
            mwbd[TOFF[i] + t, 0:64, 0:64] = q
            mwbd[TOFF[i] + t, 64:128, 64:128] = q
        cgt = np.einsum("oit,i->ot", cw, gb)  # [o, k]
        g_ = np.zeros((C, W), f32)
        for w in range(W):
            for t in range(k):
                if 0 <= w + t - pad < W:
                    g_[:, w] += cgt[:, t]
        g_ += cb[:, None]
        gbcs.append(g_)

    shared["mwbd"] = mwbd.astype(bf16)
    # gbc0/gbc2: NP layout rows, free (c, w), added via rank-1 matmul
    shared["gbc0r"] = gbcs[0].reshape(1, C * W).astype(bf16)
    shared["gbc2r"] = gbcs[2].reshape(1, C * W).astype(bf16)
    # gbc1: CP layout, free (n, w), partition (pl, c_out)
    m1 = np.tile(gbcs[1][:, None, :], (1, N, 1)).reshape(C, N * W)
    shared["gbc1f"] = np.vstack([m1, m1])
    shared["identb"] = np.eye(64).astype(bf16)

    emb_w = np.asarray(inputs["emb_w"], f32)
    emb_b = np.asarray(inputs["emb_b"], f32)
    lout_w = np.asarray(inputs["lout_w"], f32)

    in_maps = []
    for c_ in range(NCORES):
        xc = x[c_ * BL:(c_ + 1) * BL]
        m = dict(shared)
        m["x1"] = np.ascontiguousarray(
            xc.transpose(1, 0, 2).reshape(1, W * SEQ)).astype(bf16)
        h0 = xc[..., None] * emb_w + emb_b                  # [8, 48, 64, 64]
        hcp_h = np.zeros((2, 64, BP, N, WP), f32)
        hsrc = h0.reshape(2, BP, W, N, C).transpose(0, 4, 1, 3, 2)
        hcp_h[:, :, :, :, 3:3 + W] = hsrc
        m["h0cp"] = np.ascontiguousarray(hcp_h.reshape(128, CPF)).astype(bf16)
        lw = lout_w[:, c_ * KCH:(c_ + 1) * KCH]
        m["lwT"] = np.ascontiguousarray(lw.T).astype(bf16)
        in_maps.append(m)
    return in_maps


def kernel_with_stats(**inputs):
    global _NC
    if _NC is None:
        _NC = _build()
    in_maps = _host_prep(inputs)
    trace = os.environ.get("KERNEL_TRACE", "") == "1"
    res = run_bass_kernel_spmd(_NC, in_maps, core_ids=list(range(NCORES)), trace=trace)
    out = res.results[0]["out"] + np.asarray(inputs["lout_b"], np.float32)[None, :]
    return out.reshape(B, HOR, N).astype(np.float32), res


def kernel(**inputs):
    o, _ = kernel_with_stats(**inputs)
    return o
